# revision 26
# baseline (speedup 1.0000x reference)
"""Bass/Trainium2 kernel for nn_Attention_19481971654841.

Full attention block: q/k/v proj + per-head RMSNorm(q,k) + RoPE + causal GQA
SDPA + o_proj.  B=2, L=2048, D=1024, H=16, KVH=8, HD=128.

Sharding (8 NeuronCores): data-parallel over batch (2 groups of 4 cores) x
4-way tensor-parallel over heads inside each group.  Core c handles batch
c//4 and q-heads [4g:4g+4), kv-heads [2g:2g+2) with g = c%4.  Each core
produces a partial [L, D] o_proj contribution (bf16); host sums the 4
partials per batch in fp32.

Per-core dataflow (all matmuls bf16, fp32 PSUM accumulate), in three strict
phases chosen so the ACT engine never thrashes activation tables (Sqrt and
Exp live in different tables; a switch costs 1.3us).  Engine choices are
HW-calibrated: gpsimd/Pool ops run ~2.5x their cost model on HW, DVE
Reciprocal and scalar_tensor_tensor ~2-3x, so everything hot sits on
PE/ACT/DVE-tensor-tensor which measure at model rate (PE sustains ~2.0GHz
vs the model's 2.4).

Phase 1 - projections + norm + rope (PE-bound):
  - Q^T/K^T head-major [HD=128 part, T] (weights stationary on PE); V
    token-major via X-stationary matmuls.
  - RMSNorm: sum-of-squares via all-ones matmul (partition-reduce broadcast
    to 128 rows), ACT Sqrt (batched - one table load), one DVE reciprocal;
    the norm weights AND the 1/sqrt(HD) score scale are folded host-side
    into per-plane rope tables (ctab = w*cos, stab = swap(w)*sin), so
    normalize+rope is 5 plain DVE tensor-tensor ops per unit.
  - rotate-half = partition swap of the raw projection by SBUF->SBUF DMA,
    issued right after the PSUM drain so it runs concurrently with the norm
    chain; swap(q*r) = swap(q)*r since r is per-token.
  - DGE queue split: SP ring carries bulk streams (xt, wv, wo, output),
    ACT ring the small early loads, so swaps never queue behind megabytes.

Phase 2 - attention, S^T orientation, one global software pipeline:
  scores[kv, q] = K_hm^T . Q_hm (from the exact causal start; "stop" flags
  are a sim-only concept so columns may end their accumulation early with
  skip_group_check) -> ACT Exp (one table load) -> P^T bf16; causal
  diagonal zeroed post-exp by a DVE multiply with an upper-triangular 0/1
  block (PE-folded mask matmuls cost ~50us on HW: stationary swaps inside
  an accumulation chain stall the PE); softmax denominator via all-ones
  matmul; PV accumulates attn^T[hd, q] directly.  Scores+exp of pair i+1
  are emitted before denom/PV of pair i so PE never waits on ACT; ps_o/ps_d
  PSUM accumulators are allocated lazily so the single pd/pacc buffers
  never race across unit boundaries.

Phase 3 - o_proj tail: attn^T slices stationary, ACT (idle here; Copy is
  in the Exp table) drains PSUM to bf16, DMA ships to DRAM, double-buffered.
"""

import math
from contextlib import ExitStack

import numpy as np
import ml_dtypes

import concourse.bass as bass
import concourse.mybir as mybir
import concourse.tile as tile

F32 = mybir.dt.float32
BF16 = mybir.dt.bfloat16
AF = mybir.ActivationFunctionType
ALU = mybir.AluOpType

# problem constants (per spec; hardcoded — kernel.py must be self-contained)
B, L, D = 2, 2048, 1024
H, KVH, HD = 16, 8, 128
EPS = 1e-6
ROPE_BASE = 1000000.0

# per-core constants
NCORES = 8
TPG = 4                 # tensor-parallel group size (cores per batch)
T = L                   # tokens per core (one batch element)
NQ = H // TPG           # 4 q heads per core
NKV = KVH // TPG        # 2 kv heads per core
DCH = D // 128          # 8 input-dim chunks
QT = 1024               # q tile width (PSUM tile [128, 1024] f32 = 2 banks)
NQT = T // QT           # 2 q tiles
NCH = T // 128          # 16 token chunks


def _chunks512(c0, end):
    """bank-aligned matmul col chunks covering [c0, end)"""
    out = []
    n0 = c0
    while n0 < end:
        ne = min(end, (n0 // 512 + 1) * 512)
        out.append((n0, ne))
        n0 = ne
    return out


def build_nc(nrep=1, parts="full"):
    """nrep > 1 replicates the whole kernel body (same SBUF buffers, fresh
    DRAM loads) for benchmarking: per-rep steady-state time = true HW exec
    time with dispatch overhead amortized away.  kernel() always uses
    nrep=1."""
    nc = bass.Bass()

    xt_d = nc.dram_tensor("xt", [DCH, 128, T], BF16, kind="ExternalInput")
    wq_d = nc.dram_tensor("wq", [DCH, 128, NQ * HD], BF16, kind="ExternalInput")
    wk_d = nc.dram_tensor("wk", [DCH, 128, NKV * HD], BF16, kind="ExternalInput")
    wv_d = nc.dram_tensor("wv", [DCH, 128, NKV * HD], BF16, kind="ExternalInput")
    wo_d = nc.dram_tensor("wo", [NQ, 128, D], BF16, kind="ExternalInput")
    ctabq_d = nc.dram_tensor("ctabq", [128, T], BF16, kind="ExternalInput")
    stabq_d = nc.dram_tensor("stabq", [128, T], BF16, kind="ExternalInput")
    ctabk_d = nc.dram_tensor("ctabk", [128, T], BF16, kind="ExternalInput")
    stabk_d = nc.dram_tensor("stabk", [128, T], BF16, kind="ExternalInput")
    ones_d = nc.dram_tensor("ones", [128, 128], BF16, kind="ExternalInput")
    tri01_d = nc.dram_tensor("tri01", [128, 128], BF16, kind="ExternalInput")
    out_d = nc.dram_tensor("out", [NCH, 128, D], BF16, kind="ExternalOutput")

    with tile.TileContext(nc) as tc, ExitStack() as ctx:
        sing = ctx.enter_context(tc.tile_pool(name="sing", bufs=1))
        trans = ctx.enter_context(tc.tile_pool(name="trans", bufs=2))
        pts = ctx.enter_context(tc.tile_pool(name="pts", bufs=5))
        psum = ctx.enter_context(tc.tile_pool(name="psum", bufs=1, space="PSUM"))

        for _rep in range(nrep):
            # ---- persistent loads (wk + xt chunk 0 first so PE starts early)
            # Queue split: SP-DGE carries the bulk streams (xt, wv, wo and
            # later the output), ACT-DGE carries the small early loads plus
            # the per-unit rotate-half swaps, so swaps never queue behind
            # megabytes of input.
            wk = sing.tile([128, DCH, NKV * HD], BF16, tag="wk")
            nc.scalar.dma_start(out=wk, in_=wk_d.rearrange("d p f -> p d f"))
            xt = sing.tile([128, DCH, T], BF16, tag="xt")
            for d in range(DCH):
                nc.sync.dma_start(out=xt[:, d, :], in_=xt_d[d])
            wq = sing.tile([128, DCH, NQ * HD], BF16, tag="wq")
            wv = sing.tile([128, DCH, NKV * HD], BF16, tag="wv")
            nc.scalar.dma_start(out=wq, in_=wq_d.rearrange("d p f -> p d f"))
            nc.sync.dma_start(out=wv, in_=wv_d.rearrange("d p f -> p d f"))
            wo = sing.tile([128, NQ, D], BF16, tag="wo")
            nc.sync.dma_start(out=wo, in_=wo_d.rearrange("h p f -> p h f"))
            ctabq = sing.tile([128, T], BF16, tag="ctabq")
            stabq = sing.tile([128, T], BF16, tag="stabq")
            ctabk = sing.tile([128, T], BF16, tag="ctabk")
            stabk = sing.tile([128, T], BF16, tag="stabk")
            nc.scalar.dma_start(out=ctabk, in_=ctabk_d[:, :])
            nc.scalar.dma_start(out=stabk, in_=stabk_d[:, :])
            nc.scalar.dma_start(out=ctabq, in_=ctabq_d[:, :])
            nc.scalar.dma_start(out=stabq, in_=stabq_d[:, :])
            ones = sing.tile([128, 128], BF16, tag="ones")
            tri01 = sing.tile([128, 128], BF16, tag="tri01")
            nc.scalar.dma_start(out=ones, in_=ones_d[:, :])
            nc.scalar.dma_start(out=tri01, in_=tri01_d[:, :])
            epsb = sing.tile([128, 1], F32, tag="epsb")
            nc.vector.memset(epsb, EPS)

            # ---- persistent plane outputs -------------------------------
            khm = [sing.tile([128, T], BF16, tag=f"khm{i}", name=f"khm{i}")
                   for i in range(NKV)]
            qhm = [sing.tile([128, T], BF16, tag=f"qhm{i}", name=f"qhm{i}")
                   for i in range(NQ)]
            vsb = sing.tile([128, NKV, T], BF16, tag="vsb")
            attn = [sing.tile([128, T], BF16, tag=f"attn{i}", name=f"attn{i}")
                    for i in range(NQ)]
            # per-token 1/rms(k) in token-major layout [128 tok, NCH blocks],
            # applied as the Exp per-partition scale (k-norm commutes with
            # rope and the score matmul)
            rkss = [sing.tile([128, NCH], F32, tag=f"rkss{i}", name=f"rkss{i}")
                    for i in range(NKV)]
            rkT = [sing.tile([128, NCH], F32, tag=f"rkT{i}", name=f"rkT{i}")
                   for i in range(NKV)]

            # ---- phase 1: projections + norm + rope ---------------------
            # Unit = (plane, half).  Front: projection matmuls + PSUM drain
            # to bf16 + square.  Back: sum-of-squares ones-matmul, ACT Sqrt,
            # DVE reciprocal + fused scale, rope (DMA partition swap + DVE
            # muls + Pool add).
            def unit_front(tabc, tabs, wmat, fslice, half):
                qp = psum.tile([128, QT], F32, tag="pp", bufs=2, name="qp")
                for d in range(DCH):
                    for n0 in range(0, QT, 512):
                        nc.tensor.matmul(
                            qp[:, n0:n0 + 512],
                            lhsT=wmat[:, d, fslice],
                            rhs=xt[:, d, half * QT + n0: half * QT + n0 + 512],
                            start=(d == 0), stop=(d == DCH - 1),
                        )
                # drain qp on ACT (Square + Copy live in the Sqrt table, so
                # phase 1 stays on one ACT table); DVE keeps the norm chain
                sq = trans.tile([128, QT], BF16, tag="sq", bufs=2, name="sq")
                nc.scalar.activation(out=sq, in_=qp, func=AF.Square)
                qc = trans.tile([128, QT], BF16, tag="qc", bufs=3, name="qc")
                nc.scalar.activation(out=qc, in_=qp, func=AF.Copy)
                # rotate-half swap of the raw projection, early so the DMA
                # (SP ring) runs concurrently with the norm chain
                qcsw = trans.tile([128, QT], BF16, tag="qcsw", bufs=3,
                                  name="qcsw")
                nc.sync.dma_start(out=qcsw[0:64, :], in_=qc[64:128, :])
                nc.sync.dma_start(out=qcsw[64:128, :], in_=qc[0:64, :])
                return (tabc, tabs, half, qc, qcsw, sq)

            def unit_back(plane_out, kvi, st):
                tabc, tabs, half, qc, qcsw, sq = st
                cs = slice(half * QT, half * QT + QT)
                if kvi is not None:
                    # k plane: skip per-element normalization; compute rk
                    # token-major (8 single-column matmuls contracting the
                    # head dim) for the Exp scale, and rope the raw k
                    rkps = psum.tile([128, 8], F32, tag="pacc", bufs=1,
                                     name="rkps")
                    for jj in range(8):
                        nc.tensor.matmul(rkps[:, jj:jj + 1],
                                         lhsT=sq[:, 128 * jj:128 * jj + 128],
                                         rhs=ones[:, 0:1],
                                         start=True, stop=True)
                    nc.scalar.activation(
                        out=rkss[kvi][:, 8 * half:8 * half + 8], in_=rkps,
                        func=AF.Sqrt, scale=1.0 / HD, bias=epsb)
                    if half == 1:
                        nc.vector.reciprocal(out=rkT[kvi], in_=rkss[kvi])
                    mc = trans.tile([128, QT], BF16, tag="mc", name="mc")
                    nc.vector.tensor_mul(mc, qc, tabc[:, cs])
                    msw = trans.tile([128, QT], BF16, tag="msw", name="msw")
                    nc.vector.tensor_mul(msw, qcsw, tabs[:, cs])
                    nc.vector.tensor_add(plane_out[:, cs], mc, msw)
                    return
                ssq = psum.tile([128, QT], F32, tag="pd", bufs=1, name="ssq")
                for n0 in range(0, QT, 512):
                    nc.tensor.matmul(ssq[:, n0:n0 + 512], lhsT=ones,
                                     rhs=sq[:, n0:n0 + 512],
                                     start=True, stop=True)
                ss = trans.tile([128, QT], F32, tag="ss", name="ss")
                nc.scalar.activation(out=ss, in_=ssq, func=AF.Sqrt,
                                     scale=1.0 / HD, bias=epsb)
                if parts == "p1c":
                    return
                rr = trans.tile([128, QT], F32, tag="rr", name="rr")
                nc.vector.reciprocal(out=rr, in_=ss)
                qn = trans.tile([128, QT], BF16, tag="qn", name="qn")
                nc.vector.tensor_mul(qn, qc, rr)
                if parts == "p1b":
                    return
                qnsw = trans.tile([128, QT], BF16, tag="qnsw", name="qnsw")
                nc.vector.tensor_mul(qnsw, qcsw, rr)
                mc = trans.tile([128, QT], BF16, tag="mc", name="mc")
                nc.vector.tensor_mul(mc, qn, tabc[:, cs])
                msw = trans.tile([128, QT], BF16, tag="msw", name="msw")
                nc.vector.tensor_mul(msw, qnsw, tabs[:, cs])
                nc.vector.tensor_add(plane_out[:, cs], mc, msw)

            units = [(khm[i], i, ctabk, stabk, wk,
                      slice(i * HD, (i + 1) * HD), half)
                     for i in range(NKV) for half in range(NQT)]
            units += [(qhm[i], None, ctabq, stabq, wq,
                      slice(i * HD, (i + 1) * HD), half)
                      for i in range(NQ) for half in range(NQT)]
            pend = []
            for plane, kvi, tabc, tabs, wmat, fsl, half in units:
                st = unit_front(tabc, tabs, wmat, fsl, half)
                pend.append((plane, kvi, st))
                if len(pend) > 1:
                    unit_back(*pend.pop(0))
            while pend:
                unit_back(*pend.pop(0))

            # V projection: token-major via X-stationary matmuls
            for c in range(NCH):
                vp = psum.tile([128, NKV * HD], F32, tag="pacc", bufs=1,
                               name="vp")
                for d in range(DCH):
                    nc.tensor.matmul(
                        vp, lhsT=xt[:, d, c * 128:(c + 1) * 128],
                        rhs=wv[:, d, :], start=(d == 0), stop=(d == DCH - 1))
                nc.scalar.activation(
                    out=vsb[:, :, c * 128:(c + 1) * 128],
                    in_=vp.rearrange("p (k t) -> p k t", k=NKV),
                    func=AF.Copy)

            if parts.startswith("p1"):
                dummy = pts.tile([128, D], BF16, tag="ob", bufs=2, name="dummy")
                nc.vector.memset(dummy, 0.0)
                nc.sync.dma_start(out=out_d[0], in_=dummy)
                continue
            # ---- phase 2: attention, one global software pipeline -------
            # Block = (h, iqt, j).  Pairs (j, j+1) within a (h, iqt) unit;
            # scores+exp of pair i+1 are emitted before denom/PV of pair i
            # so PE never waits on ACT.  ps_o/ps_d live per (h, iqt).
            st_ctx = {}

            def scores(h, iqt, j):
                kv = h // 2
                c0 = max(0, 128 * j - QT * iqt)
                ks = slice(128 * j, 128 * j + 128)
                ps_s = psum.tile([128, QT], F32, tag="pp", bufs=2, name="ps_s")
                for n0, ne in _chunks512(c0, QT):
                    nc.tensor.matmul(
                        ps_s[:, n0:ne], lhsT=khm[kv][:, ks],
                        rhs=qhm[h][:, iqt * QT + n0: iqt * QT + ne],
                        start=True, stop=True)
                pt = pts.tile([128, QT], BF16, tag="pt", name="pt")
                nc.scalar.activation(out=pt[:, c0:QT], in_=ps_s[:, c0:QT],
                                     func=AF.Exp,
                                     scale=rkT[kv][:, j:j + 1])
                if j >= 8 * iqt:
                    # causal diagonal: zero the strictly-lower block entries
                    # on DVE post-exp (PE fold and Pool both measured slower
                    # on HW)
                    nc.vector.tensor_mul(pt[:, c0:c0 + 128],
                                         pt[:, c0:c0 + 128], tri01)
                return j, c0, pt

            def denoms(h, iqt, st):
                # streams pt from the exact causal start; "stop" is a no-op
                # on HW, so columns whose accumulation ends early just never
                # see one (skip_group_check silences the sim's group check)
                j, c0, pt = st
                jmax = 8 * iqt + 8
                if (h, iqt, "d") not in st_ctx:
                    # lazy alloc: first write lands after the previous
                    # unit's reciprocal (same pd buffer) was emitted
                    st_ctx[(h, iqt, "d")] = psum.tile(
                        [128, QT], F32, tag="pd", bufs=1, name="ps_d")
                ps_d = st_ctx[(h, iqt, "d")]
                for n0, ne in _chunks512(c0, QT):
                    jl = min(jmax - 1, 8 * iqt + (ne - 1) // 128)
                    nc.tensor.matmul(ps_d[:, n0:ne], lhsT=ones,
                                     rhs=pt[:, n0:ne],
                                     start=(j == 0), stop=(j == jl),
                                     skip_group_check=True)

            def pvs(h, iqt, st):
                j, c0, pt = st
                jmax = 8 * iqt + 8
                kv = h // 2
                if (h, iqt, "o") not in st_ctx:
                    st_ctx[(h, iqt, "o")] = psum.tile(
                        [128, QT], F32, tag="pacc", bufs=1, name="ps_o")
                ps_o = st_ctx[(h, iqt, "o")]
                kvs = slice(128 * j, 128 * j + 128)
                for n0, ne in _chunks512(c0, QT):
                    jl = min(jmax - 1, 8 * iqt + (ne - 1) // 128)
                    nc.tensor.matmul(ps_o[:, n0:ne],
                                     lhsT=vsb[:, kv, kvs],
                                     rhs=pt[:, n0:ne],
                                     start=(j == 0), stop=(j == jl),
                                     skip_group_check=True)

            def finish_unit(h, iqt):
                # stage both PSUM accumulators to SBUF first (fast copies)
                # so the pd/pacc banks are released before the reciprocal:
                # on HW DVE Reciprocal is ~3-5us and would otherwise stall
                # the next unit's denom/PV matmuls on the bank
                ps_d = st_ctx.pop((h, iqt, "d"))
                ps_o = st_ctx.pop((h, iqt, "o"))
                od = trans.tile([128, QT], F32, tag="od", name="od")
                nc.vector.tensor_copy(od, ps_o)
                dd = trans.tile([128, QT], F32, tag="dd", name="dd")
                nc.vector.tensor_copy(dd, ps_d)
                rb = trans.tile([128, QT], F32, tag="rb", name="rb")
                nc.vector.reciprocal(out=rb, in_=dd)
                nc.vector.tensor_mul(
                    attn[h][:, iqt * QT:(iqt + 1) * QT], od, rb)

            pairs = []
            for h in range(NQ):
                for iqt in range(NQT):
                    js = list(range(8 * iqt + 8))
                    pairs += [(h, iqt, js[i], js[i + 1])
                              for i in range(0, len(js), 2)]

            prev = None
            for h, iqt, j0, j1 in pairs:
                s0 = scores(h, iqt, j0)
                s1 = scores(h, iqt, j1)
                if prev is not None:
                    ph, piqt, p0, p1 = prev
                    denoms(ph, piqt, p0)
                    denoms(ph, piqt, p1)
                    pvs(ph, piqt, p0)
                    pvs(ph, piqt, p1)
                    if p1[0] == 8 * piqt + 8 - 1:  # last pair of unit
                        finish_unit(ph, piqt)
                prev = (h, iqt, s0, s1)
            ph, piqt, p0, p1 = prev
            denoms(ph, piqt, p0)
            denoms(ph, piqt, p1)
            pvs(ph, piqt, p0)
            pvs(ph, piqt, p1)
            finish_unit(ph, piqt)

            if parts == "p12":
                dummy = pts.tile([128, D], BF16, tag="ob", bufs=2, name="dummy")
                nc.vector.memset(dummy, 0.0)
                nc.sync.dma_start(out=out_d[0], in_=dummy)
                continue
            # ---- phase 3: o_proj; ACT (idle here, Copy is in the exp
            # table) drains PSUM, DMA ships to DRAM, double-buffered ------
            for c in range(NCH):
                po = psum.tile([128, D], F32, tag="pp", bufs=2, name="po")
                ts = slice(c * 128, (c + 1) * 128)
                for hh in range(NQ):
                    for n0 in range(0, D, 512):
                        nc.tensor.matmul(po[:, n0:n0 + 512],
                                         lhsT=attn[hh][:, ts],
                                         rhs=wo[:, hh, n0:n0 + 512],
                                         start=(hh == 0), stop=(hh == NQ - 1))
                ob = pts.tile([128, D], BF16, tag="ob", bufs=2, name="ob")
                nc.scalar.activation(out=ob, in_=po, func=AF.Copy)
                nc.sync.dma_start(out=out_d[c], in_=ob)

    return nc


def legalize_waits(bir_bytes):
    """This walrus build rejects compute instructions with more than one
    sync wait.  Hoist all but one wait of each instruction into standalone
    EventSemaphore (pure wait) instructions on the same engine queue, which
    is semantically identical (in-order engine queues)."""
    import json
    m = json.loads(bir_bytes)
    n_fix = 0
    for f in m["functions"]:
        for blk in f["blocks"]:
            # drop Ldweights identical to the previously-kept one (the
            # stationary operand is still loaded; bass re-emits per matmul).
            # Safe: Ldweights carry no on_update; waits (rare) are kept.
            out0 = []
            last_key = None
            for ins in blk["instructions"]:
                if ins["opcode"] == "Ldweights":
                    si = ins.get("sync_info") or {}
                    key = json.dumps(
                        [ins.get("ins"), ins.get("outs"),
                         ins.get("perf_mode"), ins.get("tile_position")])
                    if (key == last_key and not si.get("on_wait")
                            and not si.get("on_update")):
                        continue
                    last_key = key
                out0.append(ins)
            blk["instructions"] = out0
            out = []
            for ins in blk["instructions"]:
                si = ins.get("sync_info")
                waits = (si or {}).get("on_wait") or []
                if len(waits) > 1 and ins["opcode"] != "EventSemaphore":
                    for i, w in enumerate(waits[:-1]):
                        out.append({
                            "debug": ins.get("debug", 0),
                            "engine": ins["engine"],
                            "ins": [], "outs": [],
                            "name": f"{ins['name']}-hw{i}",
                            "opcode": "EventSemaphore",
                            "sync_info": {"on_update": [], "on_wait": [w]},
                        })
                    si["on_wait"] = [waits[-1]]
                    n_fix += 1
                out.append(ins)
            blk["instructions"] = out
    return json.dumps(m).encode()


def _prep_core_inputs(c, hidden_states, position_ids, q_w, k_w, v_w, o_w,
                      q_norm_w, k_norm_w):
    b, g = divmod(c, TPG)
    bf = ml_dtypes.bfloat16
    xt = np.ascontiguousarray(
        np.asarray(hidden_states[b], np.float32).T).astype(bf).reshape(DCH, 128, T)
    wq = np.ascontiguousarray(
        np.asarray(q_w[NQ * HD * g: NQ * HD * (g + 1)], np.float32).T
    ).astype(bf).reshape(DCH, 128, NQ * HD)
    wk = np.ascontiguousarray(
        np.asarray(k_w[NKV * HD * g: NKV * HD * (g + 1)], np.float32).T
    ).astype(bf).reshape(DCH, 128, NKV * HD)
    wv = np.ascontiguousarray(
        np.asarray(v_w[NKV * HD * g: NKV * HD * (g + 1)], np.float32).T
    ).astype(bf).reshape(DCH, 128, NKV * HD)
    wo = np.ascontiguousarray(
        np.asarray(o_w[:, NQ * HD * g: NQ * HD * (g + 1)], np.float32).T
    ).astype(bf).reshape(NQ, 128, D)

    pos = np.asarray(position_ids[b], np.float64)                      # [T]
    inv = 1.0 / (ROPE_BASE ** (np.arange(0, HD, 2, dtype=np.float64) / HD))
    invf2 = np.concatenate([inv, inv])                                 # [128]
    invf2s = np.concatenate([-inv, inv])
    cos = np.cos(pos[None, :] * invf2[:, None])
    sin = np.sin(pos[None, :] * invf2s[:, None])
    # RMSNorm weights (and the 1/sqrt(HD) score scale for q) folded into
    # per-plane rope tables: plane = (qc*rr)*ctab + swap(qc*rr)*stab with
    # ctab = w*cos, stab = swap(w)*sin
    qw_vec = np.asarray(q_norm_w, np.float64) / math.sqrt(HD)
    kw_vec = np.asarray(k_norm_w, np.float64)
    swap = lambda v: np.concatenate([v[64:], v[:64]])
    ctabq = (qw_vec[:, None] * cos).astype(bf)
    stabq = (swap(qw_vec)[:, None] * sin).astype(bf)
    ctabk = (kw_vec[:, None] * cos).astype(bf)
    stabk = (swap(kw_vec)[:, None] * sin).astype(bf)

    ones = np.ones((128, 128), bf)
    tri01 = np.where(np.arange(128)[:, None] <= np.arange(128)[None, :],
                     1.0, 0.0).astype(bf)
    return dict(xt=xt, wq=wq, wk=wk, wv=wv, wo=wo, ctabq=ctabq, stabq=stabq,
                ctabk=ctabk, stabk=stabk, ones=ones, tri01=tri01)


def kernel(hidden_states, position_ids, q_w, k_w, v_w, o_w, q_norm_w,
           k_norm_w):
    from concourse.bass_utils import run_bass_kernel_spmd

    nc = build_nc()
    orig_ser = nc.to_json_bytes
    nc.to_json_bytes = lambda: legalize_waits(orig_ser())
    in_maps = [
        _prep_core_inputs(c, hidden_states, position_ids, q_w, k_w, v_w, o_w,
                          q_norm_w, k_norm_w)
        for c in range(NCORES)
    ]
    res = run_bass_kernel_spmd(nc, in_maps, list(range(NCORES))).results
    out = np.zeros((B, L, D), np.float32)
    for c in range(NCORES):
        out[c // TPG] += np.asarray(res[c]["out"], np.float32).reshape(L, D)
    return out


# revision 27
# speedup vs baseline: 1.0333x; 1.0333x over previous
"""Bass/Trainium2 kernel for nn_Attention_19481971654841.

Full attention block: q/k/v proj + per-head RMSNorm(q,k) + RoPE + causal GQA
SDPA + o_proj.  B=2, L=2048, D=1024, H=16, KVH=8, HD=128.

Sharding (8 NeuronCores): data-parallel over batch (2 groups of 4 cores) x
4-way tensor-parallel over heads inside each group.  Core c handles batch
c//4 and q-heads [4g:4g+4), kv-heads [2g:2g+2) with g = c%4.  Each core
produces a partial [L, D] o_proj contribution (bf16); host sums the 4
partials per batch in fp32.

Per-core dataflow (all matmuls bf16, fp32 PSUM accumulate), in three strict
phases chosen so the ACT engine never thrashes activation tables (Sqrt and
Exp live in different tables; a switch costs 1.3us).  Engine choices are
HW-calibrated: gpsimd/Pool ops run ~2.5x their cost model on HW, DVE
Reciprocal and scalar_tensor_tensor ~2-3x, so everything hot sits on
PE/ACT/DVE-tensor-tensor which measure at model rate (PE sustains ~2.0GHz
vs the model's 2.4).

Phase 1 - projections + norm + rope (PE-bound):
  - Q^T/K^T head-major [HD=128 part, T] (weights stationary on PE); V
    token-major via X-stationary matmuls.
  - RMSNorm: sum-of-squares via all-ones matmul (partition-reduce broadcast
    to 128 rows), ACT Sqrt (batched - one table load), one DVE reciprocal;
    the norm weights AND the 1/sqrt(HD) score scale are folded host-side
    into per-plane rope tables (ctab = w*cos, stab = swap(w)*sin), so
    normalize+rope is 5 plain DVE tensor-tensor ops per unit.
  - rotate-half = partition swap of the raw projection by SBUF->SBUF DMA,
    issued right after the PSUM drain so it runs concurrently with the norm
    chain; swap(q*r) = swap(q)*r since r is per-token.
  - DGE queue split: SP ring carries bulk streams (xt, wv, wo, output),
    ACT ring the small early loads, so swaps never queue behind megabytes.

Phase 2 - attention, S^T orientation, one global software pipeline:
  scores[kv, q] = K_hm^T . Q_hm (from the exact causal start; "stop" flags
  are a sim-only concept so columns may end their accumulation early with
  skip_group_check) -> ACT Exp (one table load) -> P^T bf16; causal
  diagonal zeroed post-exp by a DVE multiply with an upper-triangular 0/1
  block (PE-folded mask matmuls cost ~50us on HW: stationary swaps inside
  an accumulation chain stall the PE); softmax denominator via all-ones
  matmul; PV accumulates attn^T[hd, q] directly.  Scores+exp of pair i+1
  are emitted before denom/PV of pair i so PE never waits on ACT; ps_o/ps_d
  PSUM accumulators are allocated lazily so the single pd/pacc buffers
  never race across unit boundaries.

Phase 3 - o_proj tail: attn^T slices stationary, ACT (idle here; Copy is
  in the Exp table) drains PSUM to bf16, DMA ships to DRAM, double-buffered.
"""

import math
from contextlib import ExitStack

import numpy as np
import ml_dtypes

import concourse.bass as bass
import concourse.mybir as mybir
import concourse.tile as tile

F32 = mybir.dt.float32
BF16 = mybir.dt.bfloat16
AF = mybir.ActivationFunctionType
ALU = mybir.AluOpType

# problem constants (per spec; hardcoded — kernel.py must be self-contained)
B, L, D = 2, 2048, 1024
H, KVH, HD = 16, 8, 128
EPS = 1e-6
ROPE_BASE = 1000000.0

# per-core constants
NCORES = 8
TPG = 4                 # tensor-parallel group size (cores per batch)
T = L                   # tokens per core (one batch element)
NQ = H // TPG           # 4 q heads per core
NKV = KVH // TPG        # 2 kv heads per core
DCH = D // 128          # 8 input-dim chunks
QT = 1024               # q tile width (PSUM tile [128, 1024] f32 = 2 banks)
NQT = T // QT           # 2 q tiles
NCH = T // 128          # 16 token chunks


def _chunks512(c0, end):
    """bank-aligned matmul col chunks covering [c0, end)"""
    out = []
    n0 = c0
    while n0 < end:
        ne = min(end, (n0 // 512 + 1) * 512)
        out.append((n0, ne))
        n0 = ne
    return out


def build_nc(nrep=1, parts="full"):
    """nrep > 1 replicates the whole kernel body (same SBUF buffers, fresh
    DRAM loads) for benchmarking: per-rep steady-state time = true HW exec
    time with dispatch overhead amortized away.  kernel() always uses
    nrep=1."""
    nc = bass.Bass()

    xt_d = nc.dram_tensor("xt", [DCH, 128, T], BF16, kind="ExternalInput")
    wq_d = nc.dram_tensor("wq", [DCH, 128, NQ * HD], BF16, kind="ExternalInput")
    wk_d = nc.dram_tensor("wk", [DCH, 128, NKV * HD], BF16, kind="ExternalInput")
    wv_d = nc.dram_tensor("wv", [DCH, 128, NKV * HD], BF16, kind="ExternalInput")
    wo_d = nc.dram_tensor("wo", [NQ, 128, D], BF16, kind="ExternalInput")
    ctabq_d = nc.dram_tensor("ctabq", [128, T], BF16, kind="ExternalInput")
    stabq_d = nc.dram_tensor("stabq", [128, T], BF16, kind="ExternalInput")
    ctabk_d = nc.dram_tensor("ctabk", [128, T], BF16, kind="ExternalInput")
    stabk_d = nc.dram_tensor("stabk", [128, T], BF16, kind="ExternalInput")
    ones_d = nc.dram_tensor("ones", [128, 128], BF16, kind="ExternalInput")
    tri01_d = nc.dram_tensor("tri01", [128, 128], BF16, kind="ExternalInput")
    out_d = nc.dram_tensor("out", [NCH, 128, D], BF16, kind="ExternalOutput")

    with tile.TileContext(nc) as tc, ExitStack() as ctx:
        sing = ctx.enter_context(tc.tile_pool(name="sing", bufs=1))
        trans = ctx.enter_context(tc.tile_pool(name="trans", bufs=2))
        pts = ctx.enter_context(tc.tile_pool(name="pts", bufs=4))
        psum = ctx.enter_context(tc.tile_pool(name="psum", bufs=1, space="PSUM"))

        for _rep in range(nrep):
            # ---- persistent loads (wk + xt chunk 0 first so PE starts early)
            # Queue split: SP-DGE carries the bulk streams (xt, wv, wo and
            # later the output), ACT-DGE carries the small early loads plus
            # the per-unit rotate-half swaps, so swaps never queue behind
            # megabytes of input.
            wk = sing.tile([128, DCH, NKV * HD], BF16, tag="wk")
            nc.scalar.dma_start(out=wk, in_=wk_d.rearrange("d p f -> p d f"))
            xt = sing.tile([128, DCH, T], BF16, tag="xt")
            for d in range(DCH):
                nc.sync.dma_start(out=xt[:, d, :], in_=xt_d[d])
            wq = sing.tile([128, DCH, NQ * HD], BF16, tag="wq")
            wv = sing.tile([128, DCH, NKV * HD], BF16, tag="wv")
            nc.scalar.dma_start(out=wq, in_=wq_d.rearrange("d p f -> p d f"))
            nc.sync.dma_start(out=wv, in_=wv_d.rearrange("d p f -> p d f"))
            wo = sing.tile([128, NQ, D], BF16, tag="wo")
            nc.sync.dma_start(out=wo, in_=wo_d.rearrange("h p f -> p h f"))
            ctabq = sing.tile([128, T], BF16, tag="ctabq")
            stabq = sing.tile([128, T], BF16, tag="stabq")
            ctabk = sing.tile([128, T], BF16, tag="ctabk")
            stabk = sing.tile([128, T], BF16, tag="stabk")
            nc.scalar.dma_start(out=ctabk, in_=ctabk_d[:, :])
            nc.scalar.dma_start(out=stabk, in_=stabk_d[:, :])
            nc.scalar.dma_start(out=ctabq, in_=ctabq_d[:, :])
            nc.scalar.dma_start(out=stabq, in_=stabq_d[:, :])
            ones = sing.tile([128, 128], BF16, tag="ones")
            tri01 = sing.tile([128, 128], BF16, tag="tri01")
            nc.scalar.dma_start(out=ones, in_=ones_d[:, :])
            nc.scalar.dma_start(out=tri01, in_=tri01_d[:, :])
            epsb = sing.tile([128, 1], F32, tag="epsb")
            nc.vector.memset(epsb, EPS)

            # ---- persistent plane outputs -------------------------------
            khm = [sing.tile([128, T], BF16, tag=f"khm{i}", name=f"khm{i}")
                   for i in range(NKV)]
            qhm = [sing.tile([128, T], BF16, tag=f"qhm{i}", name=f"qhm{i}")
                   for i in range(NQ)]
            vsb = sing.tile([128, NKV, T], BF16, tag="vsb")
            attn = [sing.tile([128, T], BF16, tag=f"attn{i}", name=f"attn{i}")
                    for i in range(NQ)]
            # per-token 1/rms(k) in token-major layout [128 tok, NCH blocks],
            # applied as the Exp per-partition scale (k-norm commutes with
            # rope and the score matmul)
            rkss = [sing.tile([128, NCH], F32, tag=f"rkss{i}", name=f"rkss{i}")
                    for i in range(NKV)]
            rkT = [sing.tile([128, NCH], F32, tag=f"rkT{i}", name=f"rkT{i}")
                   for i in range(NKV)]

            # ---- phase 1: projections + norm + rope ---------------------
            # Unit = (plane, half).  Front: projection matmuls + PSUM drain
            # to bf16 + square.  Back: sum-of-squares ones-matmul, ACT Sqrt,
            # DVE reciprocal + fused scale, rope (DMA partition swap + DVE
            # muls + Pool add).
            def unit_front(tabc, tabs, wmat, fslice, half):
                qp = psum.tile([128, QT], F32, tag="pp", bufs=2, name="qp")
                for d in range(DCH):
                    for n0 in range(0, QT, 512):
                        nc.tensor.matmul(
                            qp[:, n0:n0 + 512],
                            lhsT=wmat[:, d, fslice],
                            rhs=xt[:, d, half * QT + n0: half * QT + n0 + 512],
                            start=(d == 0), stop=(d == DCH - 1),
                        )
                # drain qp on ACT (Square + Copy live in the Sqrt table, so
                # phase 1 stays on one ACT table); DVE keeps the norm chain
                sq = trans.tile([128, QT], BF16, tag="sq", bufs=2, name="sq")
                nc.scalar.activation(out=sq, in_=qp, func=AF.Square)
                qc = trans.tile([128, QT], BF16, tag="qc", bufs=3, name="qc")
                nc.scalar.activation(out=qc, in_=qp, func=AF.Copy)
                # rotate-half swap of the raw projection, early so the DMA
                # (SP ring) runs concurrently with the norm chain
                qcsw = trans.tile([128, QT], BF16, tag="qcsw", bufs=3,
                                  name="qcsw")
                nc.sync.dma_start(out=qcsw[0:64, :], in_=qc[64:128, :])
                nc.sync.dma_start(out=qcsw[64:128, :], in_=qc[0:64, :])
                return (tabc, tabs, half, qc, qcsw, sq)

            def unit_back(plane_out, kvi, st):
                tabc, tabs, half, qc, qcsw, sq = st
                cs = slice(half * QT, half * QT + QT)
                if kvi is not None:
                    # k plane: skip per-element normalization; compute rk
                    # token-major (8 single-column matmuls contracting the
                    # head dim) for the Exp scale, and rope the raw k
                    rkps = psum.tile([128, 8], F32, tag="pacc", bufs=1,
                                     name="rkps")
                    for jj in range(8):
                        nc.tensor.matmul(rkps[:, jj:jj + 1],
                                         lhsT=sq[:, 128 * jj:128 * jj + 128],
                                         rhs=ones[:, 0:1],
                                         start=True, stop=True)
                    nc.scalar.activation(
                        out=rkss[kvi][:, 8 * half:8 * half + 8], in_=rkps,
                        func=AF.Sqrt, scale=1.0 / HD, bias=epsb)
                    if half == 1:
                        nc.vector.reciprocal(out=rkT[kvi], in_=rkss[kvi])
                    mc = trans.tile([128, QT], BF16, tag="mc", name="mc")
                    nc.vector.tensor_mul(mc, qc, tabc[:, cs])
                    msw = trans.tile([128, QT], BF16, tag="msw", name="msw")
                    nc.vector.tensor_mul(msw, qcsw, tabs[:, cs])
                    nc.vector.tensor_add(plane_out[:, cs], mc, msw)
                    return
                ssq = psum.tile([128, QT], F32, tag="pd", bufs=1, name="ssq")
                for n0 in range(0, QT, 512):
                    nc.tensor.matmul(ssq[:, n0:n0 + 512], lhsT=ones,
                                     rhs=sq[:, n0:n0 + 512],
                                     start=True, stop=True)
                ss = trans.tile([128, QT], F32, tag="ss", name="ss")
                nc.scalar.activation(out=ss, in_=ssq, func=AF.Sqrt,
                                     scale=1.0 / HD, bias=epsb)
                if parts == "p1c":
                    return
                rr = trans.tile([128, QT], F32, tag="rr", name="rr")
                nc.vector.reciprocal(out=rr, in_=ss)
                qn = trans.tile([128, QT], BF16, tag="qn", name="qn")
                nc.vector.tensor_mul(qn, qc, rr)
                if parts == "p1b":
                    return
                qnsw = trans.tile([128, QT], BF16, tag="qnsw", name="qnsw")
                nc.vector.tensor_mul(qnsw, qcsw, rr)
                mc = trans.tile([128, QT], BF16, tag="mc", name="mc")
                nc.vector.tensor_mul(mc, qn, tabc[:, cs])
                msw = trans.tile([128, QT], BF16, tag="msw", name="msw")
                nc.vector.tensor_mul(msw, qnsw, tabs[:, cs])
                nc.vector.tensor_add(plane_out[:, cs], mc, msw)

            units = [(khm[i], i, ctabk, stabk, wk,
                      slice(i * HD, (i + 1) * HD), half)
                     for i in range(NKV) for half in range(NQT)]
            units += [(qhm[i], None, ctabq, stabq, wq,
                      slice(i * HD, (i + 1) * HD), half)
                      for i in range(NQ) for half in range(NQT)]
            pend = []
            for plane, kvi, tabc, tabs, wmat, fsl, half in units:
                st = unit_front(tabc, tabs, wmat, fsl, half)
                pend.append((plane, kvi, st))
                if len(pend) > 1:
                    unit_back(*pend.pop(0))
            while pend:
                unit_back(*pend.pop(0))

            # V projection: token-major via X-stationary matmuls
            for c in range(NCH):
                vp = psum.tile([128, NKV * HD], F32, tag="pacc", bufs=1,
                               name="vp")
                for d in range(DCH):
                    nc.tensor.matmul(
                        vp, lhsT=xt[:, d, c * 128:(c + 1) * 128],
                        rhs=wv[:, d, :], start=(d == 0), stop=(d == DCH - 1))
                nc.scalar.activation(
                    out=vsb[:, :, c * 128:(c + 1) * 128],
                    in_=vp.rearrange("p (k t) -> p k t", k=NKV),
                    func=AF.Copy)

            if parts.startswith("p1"):
                dummy = pts.tile([128, D], BF16, tag="ob", bufs=2, name="dummy")
                nc.vector.memset(dummy, 0.0)
                nc.sync.dma_start(out=out_d[0], in_=dummy)
                continue
            # ---- phase 2: attention, one global software pipeline -------
            # Block = (h, iqt, j).  Pairs (j, j+1) within a (h, iqt) unit;
            # scores+exp of pair i+1 are emitted before denom/PV of pair i
            # so PE never waits on ACT.  ps_o/ps_d live per (h, iqt).
            st_ctx = {}

            def scores(h, iqt, j):
                kv = h // 2
                c0 = max(0, 128 * j - QT * iqt)
                ks = slice(128 * j, 128 * j + 128)
                ps_s = psum.tile([128, QT], F32, tag="pp", bufs=2, name="ps_s")
                for n0, ne in _chunks512(c0, QT):
                    nc.tensor.matmul(
                        ps_s[:, n0:ne], lhsT=khm[kv][:, ks],
                        rhs=qhm[h][:, iqt * QT + n0: iqt * QT + ne],
                        start=True, stop=True)
                pt = pts.tile([128, QT], BF16, tag="pt", name="pt")
                nc.scalar.activation(out=pt[:, c0:QT], in_=ps_s[:, c0:QT],
                                     func=AF.Exp,
                                     scale=rkT[kv][:, j:j + 1])
                if j >= 8 * iqt:
                    # causal diagonal: zero the strictly-lower block entries
                    # on DVE post-exp (PE fold and Pool both measured slower
                    # on HW)
                    nc.vector.tensor_mul(pt[:, c0:c0 + 128],
                                         pt[:, c0:c0 + 128], tri01)
                return j, c0, pt

            def denoms(h, iqt, st):
                # streams pt from the exact causal start; "stop" is a no-op
                # on HW, so columns whose accumulation ends early just never
                # see one (skip_group_check silences the sim's group check)
                j, c0, pt = st
                jmax = 8 * iqt + 8
                if (h, iqt, "d") not in st_ctx:
                    # lazy alloc: first write lands after the previous
                    # unit's reciprocal (same pd buffer) was emitted
                    st_ctx[(h, iqt, "d")] = psum.tile(
                        [128, QT], F32, tag="pd", bufs=1, name="ps_d")
                ps_d = st_ctx[(h, iqt, "d")]
                for n0, ne in _chunks512(c0, QT):
                    jl = min(jmax - 1, 8 * iqt + (ne - 1) // 128)
                    nc.tensor.matmul(ps_d[:, n0:ne], lhsT=ones,
                                     rhs=pt[:, n0:ne],
                                     start=(j == 0), stop=(j == jl),
                                     skip_group_check=True)

            def pvs(h, iqt, st):
                j, c0, pt = st
                jmax = 8 * iqt + 8
                kv = h // 2
                if (h, iqt, "o") not in st_ctx:
                    st_ctx[(h, iqt, "o")] = psum.tile(
                        [128, QT], F32, tag="pacc", bufs=1, name="ps_o")
                ps_o = st_ctx[(h, iqt, "o")]
                kvs = slice(128 * j, 128 * j + 128)
                for n0, ne in _chunks512(c0, QT):
                    jl = min(jmax - 1, 8 * iqt + (ne - 1) // 128)
                    nc.tensor.matmul(ps_o[:, n0:ne],
                                     lhsT=vsb[:, kv, kvs],
                                     rhs=pt[:, n0:ne],
                                     start=(j == 0), stop=(j == jl),
                                     skip_group_check=True)

            def finish_unit(h, iqt):
                # stage both PSUM accumulators to SBUF first (fast copies)
                # so the pd/pacc banks are released before the reciprocal:
                # on HW DVE Reciprocal is ~3-5us and would otherwise stall
                # the next unit's denom/PV matmuls on the bank
                ps_d = st_ctx.pop((h, iqt, "d"))
                ps_o = st_ctx.pop((h, iqt, "o"))
                od = trans.tile([128, QT], F32, tag="od", name="od")
                nc.vector.tensor_copy(od, ps_o)
                dd = trans.tile([128, QT], F32, tag="dd", name="dd")
                nc.vector.tensor_copy(dd, ps_d)
                rb = trans.tile([128, QT], F32, tag="rb", name="rb")
                nc.vector.reciprocal(out=rb, in_=dd)
                nc.vector.tensor_mul(
                    attn[h][:, iqt * QT:(iqt + 1) * QT], od, rb)

            pairs = []
            for h in range(NQ):
                for iqt in range(NQT):
                    js = list(range(8 * iqt + 8))
                    pairs += [(h, iqt, js[i], js[i + 1])
                              for i in range(0, len(js), 2)]

            prev = None
            for h, iqt, j0, j1 in pairs:
                s0 = scores(h, iqt, j0)
                s1 = scores(h, iqt, j1)
                if prev is not None:
                    ph, piqt, p0, p1 = prev
                    denoms(ph, piqt, p0)
                    denoms(ph, piqt, p1)
                    pvs(ph, piqt, p0)
                    pvs(ph, piqt, p1)
                    if p1[0] == 8 * piqt + 8 - 1:  # last pair of unit
                        finish_unit(ph, piqt)
                prev = (h, iqt, s0, s1)
            ph, piqt, p0, p1 = prev
            denoms(ph, piqt, p0)
            denoms(ph, piqt, p1)
            pvs(ph, piqt, p0)
            pvs(ph, piqt, p1)
            finish_unit(ph, piqt)

            if parts == "p12":
                dummy = pts.tile([128, D], BF16, tag="ob", bufs=2, name="dummy")
                nc.vector.memset(dummy, 0.0)
                nc.sync.dma_start(out=out_d[0], in_=dummy)
                continue
            # ---- phase 3: o_proj; ACT (idle here, Copy is in the exp
            # table) drains PSUM, DMA ships to DRAM, double-buffered ------
            for c in range(NCH):
                po = psum.tile([128, D], F32, tag="pp", bufs=2, name="po")
                ts = slice(c * 128, (c + 1) * 128)
                for hh in range(NQ):
                    for n0 in range(0, D, 512):
                        nc.tensor.matmul(po[:, n0:n0 + 512],
                                         lhsT=attn[hh][:, ts],
                                         rhs=wo[:, hh, n0:n0 + 512],
                                         start=(hh == 0), stop=(hh == NQ - 1))
                ob = pts.tile([128, D], BF16, tag="ob", bufs=2, name="ob")
                nc.scalar.activation(out=ob, in_=po, func=AF.Copy)
                nc.sync.dma_start(out=out_d[c], in_=ob)

    return nc


def legalize_waits(bir_bytes):
    """This walrus build rejects compute instructions with more than one
    sync wait.  Hoist all but one wait of each instruction into standalone
    EventSemaphore (pure wait) instructions on the same engine queue, which
    is semantically identical (in-order engine queues)."""
    import json
    m = json.loads(bir_bytes)
    n_fix = 0
    for f in m["functions"]:
        for blk in f["blocks"]:
            # drop Ldweights identical to the previously-kept one (the
            # stationary operand is still loaded; bass re-emits per matmul).
            # Safe: Ldweights carry no on_update; waits (rare) are kept.
            out0 = []
            last_key = None
            for ins in blk["instructions"]:
                if ins["opcode"] == "Ldweights":
                    si = ins.get("sync_info") or {}
                    key = json.dumps(
                        [ins.get("ins"), ins.get("outs"),
                         ins.get("perf_mode"), ins.get("tile_position")])
                    if (key == last_key and not si.get("on_wait")
                            and not si.get("on_update")):
                        continue
                    last_key = key
                out0.append(ins)
            blk["instructions"] = out0
            out = []
            for ins in blk["instructions"]:
                si = ins.get("sync_info")
                waits = (si or {}).get("on_wait") or []
                if len(waits) > 1 and ins["opcode"] != "EventSemaphore":
                    for i, w in enumerate(waits[:-1]):
                        out.append({
                            "debug": ins.get("debug", 0),
                            "engine": ins["engine"],
                            "ins": [], "outs": [],
                            "name": f"{ins['name']}-hw{i}",
                            "opcode": "EventSemaphore",
                            "sync_info": {"on_update": [], "on_wait": [w]},
                        })
                    si["on_wait"] = [waits[-1]]
                    n_fix += 1
                out.append(ins)
            blk["instructions"] = out
    return json.dumps(m).encode()


def _prep_core_inputs(c, hidden_states, position_ids, q_w, k_w, v_w, o_w,
                      q_norm_w, k_norm_w):
    b, g = divmod(c, TPG)
    bf = ml_dtypes.bfloat16
    xt = np.ascontiguousarray(
        np.asarray(hidden_states[b], np.float32).T).astype(bf).reshape(DCH, 128, T)
    wq = np.ascontiguousarray(
        np.asarray(q_w[NQ * HD * g: NQ * HD * (g + 1)], np.float32).T
    ).astype(bf).reshape(DCH, 128, NQ * HD)
    wk = np.ascontiguousarray(
        np.asarray(k_w[NKV * HD * g: NKV * HD * (g + 1)], np.float32).T
    ).astype(bf).reshape(DCH, 128, NKV * HD)
    wv = np.ascontiguousarray(
        np.asarray(v_w[NKV * HD * g: NKV * HD * (g + 1)], np.float32).T
    ).astype(bf).reshape(DCH, 128, NKV * HD)
    wo = np.ascontiguousarray(
        np.asarray(o_w[:, NQ * HD * g: NQ * HD * (g + 1)], np.float32).T
    ).astype(bf).reshape(NQ, 128, D)

    pos = np.asarray(position_ids[b], np.float64)                      # [T]
    inv = 1.0 / (ROPE_BASE ** (np.arange(0, HD, 2, dtype=np.float64) / HD))
    invf2 = np.concatenate([inv, inv])                                 # [128]
    invf2s = np.concatenate([-inv, inv])
    cos = np.cos(pos[None, :] * invf2[:, None])
    sin = np.sin(pos[None, :] * invf2s[:, None])
    # RMSNorm weights (and the 1/sqrt(HD) score scale for q) folded into
    # per-plane rope tables: plane = (qc*rr)*ctab + swap(qc*rr)*stab with
    # ctab = w*cos, stab = swap(w)*sin
    qw_vec = np.asarray(q_norm_w, np.float64) / math.sqrt(HD)
    kw_vec = np.asarray(k_norm_w, np.float64)
    swap = lambda v: np.concatenate([v[64:], v[:64]])
    ctabq = (qw_vec[:, None] * cos).astype(bf)
    stabq = (swap(qw_vec)[:, None] * sin).astype(bf)
    ctabk = (kw_vec[:, None] * cos).astype(bf)
    stabk = (swap(kw_vec)[:, None] * sin).astype(bf)

    ones = np.ones((128, 128), bf)
    tri01 = np.where(np.arange(128)[:, None] <= np.arange(128)[None, :],
                     1.0, 0.0).astype(bf)
    return dict(xt=xt, wq=wq, wk=wk, wv=wv, wo=wo, ctabq=ctabq, stabq=stabq,
                ctabk=ctabk, stabk=stabk, ones=ones, tri01=tri01)


def kernel(hidden_states, position_ids, q_w, k_w, v_w, o_w, q_norm_w,
           k_norm_w):
    from concourse.bass_utils import run_bass_kernel_spmd

    nc = build_nc()
    orig_ser = nc.to_json_bytes
    nc.to_json_bytes = lambda: legalize_waits(orig_ser())
    in_maps = [
        _prep_core_inputs(c, hidden_states, position_ids, q_w, k_w, v_w, o_w,
                          q_norm_w, k_norm_w)
        for c in range(NCORES)
    ]
    res = run_bass_kernel_spmd(nc, in_maps, list(range(NCORES))).results
    out = np.zeros((B, L, D), np.float32)
    for c in range(NCORES):
        out[c // TPG] += np.asarray(res[c]["out"], np.float32).reshape(L, D)
    return out
